# revision 9
# baseline (speedup 1.0000x reference)
"""Trainium2 Bass kernel for ConfidenceGatedSymmetryModule (RANSAC symmetry plane).

kernel(**inputs) takes FULL unsharded inputs (B=8), returns the full output
tuple (z_enc_aug, z_local, normals, offsets, confidences).

Data parallel on B: one batch per NeuronCore, 8 cores.

Per plane t the reference reflects all N points and tests, for each reflected
point, whether ANY original point lies within MATCH_THR. The [N,N] test
out[n,m] = thr2 - d2[n,m] is a K=5 matmul on PE:
  lhsT rows: [2rx, 2ry, 2rz, thr2-|r|^2, 1],  rhs rows: [px, py, pz, 1, -|p|^2]
Pruning: |refl_n - p_m| >= |sd_n + sd_m| (signed plane distances), so only m
with sd_m in [-sd_hi-δ, -sd_lo+δ] of a 128-row sd-sorted block can match.
The host gathers each block's candidate interval into a PACKED per-plane rhs
array, so window offsets are construction-time constants. Plane slots are
sorted by total width and block slots by width so the compiled widths (union
across batches) stay tight for iid batches. ~8x less PE/DVE work vs brute.

Device: per chunk (<=512 cols): matmul [5,128]^T @ [5,cw] -> PSUM,
DVE reduce_max -> one column of out. Host: max over a unit's chunk columns,
threshold/count -> confidences; |max|<eps rows re-checked exactly on host;
reference's sequential strict-> argmax; tiny 260->256 linear in numpy.
"""

import sys
import numpy as np

sys.path.insert(0, "/opt/trn_rl_repo")

MATCH_THR = np.float32(0.05)
B, N, T, D = 8, 2048, 64, 256
NBLK = N // 128          # 16 blocks of 128 reflected rows per plane
BAND = np.float32(0.061)  # sd-window half-width (> MATCH_THR + fp slack)
BORDER_EPS = 1e-4        # |thr2-d2| below this -> exact host recheck
CHUNK = 512              # matmul moving free dim / PSUM bank

_COMPILED = {}


def _thr2():
    # largest f32 x with sqrt(x) < MATCH_THR, so (d2 <= thr2) == (sqrt(d2) < thr)
    c = np.float32(MATCH_THR)
    x = np.float32(c) * np.float32(c)
    while np.sqrt(x) < c:
        x = np.nextafter(x, np.float32(np.inf), dtype=np.float32)
    while np.sqrt(x) >= c:
        x = np.nextafter(x, np.float32(-np.inf), dtype=np.float32)
    return x


def _chunk_plan(W_union):
    """Shared builder/finish layout derived from baked widths.

    Returns (col_map, rhs_off, Lmax):
      col_map: list of (s, j, rhs_lo, cw) one per out column (chunk)
      rhs_off[s][j]: start of unit (s,j)'s candidates in slot s's rhs row
      Lmax: padded rhs row length
    """
    col_map = []
    rhs_off = np.zeros((T, NBLK), np.int64)
    Lmax = 0
    for s in range(T):
        pos = 0
        for j in range(NBLK):
            w = int(W_union[s][j])
            rhs_off[s][j] = pos
            lo = 0
            while lo < w:
                cw = min(CHUNK, w - lo)
                col_map.append((s, j, pos + lo, cw))
                lo += cw
            pos += w
        Lmax = max(Lmax, pos)
    Lmax = ((Lmax + 63) // 64) * 64
    return col_map, rhs_off, Lmax


def _build_bass(W_union):
    import concourse.bacc as bacc
    import concourse.tile as tile
    from concourse import mybir

    col_map, rhs_off, Lmax = _chunk_plan(W_union)
    ncols = len(col_map)

    nc = bacc.Bacc("TRN2", target_bir_lowering=False, debug=False, num_devices=8)
    rhs_d = nc.dram_tensor("rhs", [T, 5, Lmax], mybir.dt.float32,
                           kind="ExternalInput")
    wts_d = nc.dram_tensor("wts", [T, 5, N], mybir.dt.float32,
                           kind="ExternalInput")
    out_d = nc.dram_tensor("maxv", [128, ncols], mybir.dt.float32,
                           kind="ExternalOutput")

    # group col_map by slot for the per-slot loop
    by_slot = [[] for _ in range(T)]
    for ci, (s, j, lo, cw) in enumerate(col_map):
        by_slot[s].append((ci, j, lo, cw))

    with tile.TileContext(nc) as tc:
        with (
            tc.tile_pool(name="rhs", bufs=3) as rhs_pool,
            tc.tile_pool(name="wts", bufs=3) as wts_pool,
            tc.tile_pool(name="out", bufs=1) as out_pool,
            tc.tile_pool(name="psum", bufs=8, space="PSUM") as psum_pool,
        ):
            out_sb = out_pool.tile([128, ncols], mybir.dt.float32)
            for s in range(T):
                r_sb = rhs_pool.tile([5, Lmax], mybir.dt.float32, tag="r")
                nc.sync.dma_start(r_sb[:], rhs_d[s])
                w_sb = wts_pool.tile([5, N], mybir.dt.float32, tag="w")
                nc.sync.dma_start(w_sb[:], wts_d[s])
                for ci, j, lo, cw in by_slot[s]:
                    ps = psum_pool.tile([128, CHUNK], mybir.dt.float32, tag="ps")
                    nc.tensor.matmul(
                        ps[:, :cw],
                        w_sb[:, j * 128:(j + 1) * 128],
                        r_sb[:, lo:lo + cw],
                        start=True, stop=True,
                    )
                    nc.vector.reduce_max(
                        out=out_sb[:, ci:ci + 1],
                        in_=ps[:, :cw],
                        axis=mybir.AxisListType.X,
                    )
            nc.sync.dma_start(out_d[:], out_sb[:])
    nc.compile()
    return nc


def _host_prep(points, sample_idx):
    """Per-batch host prep (no device operands yet; widths first)."""
    f32 = np.float32
    pts = points.astype(f32)                      # [N,3]
    psq = (pts * pts).sum(-1, dtype=f32)          # [N]

    q1 = pts[sample_idx[:, 0]]
    q2 = pts[sample_idx[:, 1]]
    d = q2 - q1
    nd = np.sqrt((d * d).sum(-1, dtype=f32))      # [T]
    n = d / (nd[:, None] + f32(1e-12))
    offset = (n * (q1 + q2) * f32(0.5)).sum(-1, dtype=f32)
    nn = n / (np.sqrt((n * n).sum(-1, dtype=f32))[:, None] + f32(1e-12))

    sd = pts @ nn.T - offset[None, :]             # [N,T]
    refl = pts[None] - 2.0 * sd.T[:, :, None] * nn[:, None, :]
    refl = refl.astype(f32)                       # [T,N,3]
    rsq = (refl * refl).sum(-1, dtype=f32)        # [T,N]

    ords = np.argsort(sd, axis=0, kind="stable").T  # [T,N] sd-sorted row order
    ssd = np.take_along_axis(sd.T, ords, axis=1)    # [T,N]

    degen = nd < 1e-8
    widths = np.zeros((T, NBLK), np.int64)
    los = np.zeros((T, NBLK), np.int64)
    for t in range(T):
        if degen[t]:
            continue
        blo = ssd[t].reshape(NBLK, 128)[:, 0]
        bhi = ssd[t].reshape(NBLK, 128)[:, -1]
        lo = np.searchsorted(ssd[t], -bhi - BAND, "left")
        hi = np.searchsorted(ssd[t], -blo + BAND, "right")
        los[t], widths[t] = lo, hi - lo

    wsum = widths.sum(-1)
    plane_rank = np.argsort(wsum, kind="stable")       # slot s -> plane t
    blk_rank = np.argsort(-widths, axis=-1, kind="stable")  # per plane: slot j -> block k
    # widths arranged by slot: [s][j]
    wslot = np.stack([widths[plane_rank[s]][blk_rank[plane_rank[s]]]
                      for s in range(T)])
    wslot = ((wslot + 63) // 64) * 64

    return dict(pts=pts, psq=psq, n=n, nn=nn, nd=nd, offset=offset,
                refl=refl, rsq=rsq, ords=ords, los=los, widths=widths,
                plane_rank=plane_rank, blk_rank=blk_rank, wslot=wslot,
                thr2=_thr2())


def _host_operands(prep, W_union, Lmax):
    """Build rhs [T,5,Lmax] and wts [T,5,N] for one batch given baked widths."""
    f32 = np.float32
    pts, psq = prep["pts"], prep["psq"]
    refl, rsq = prep["refl"], prep["rsq"]
    ords = prep["ords"]
    thr2 = prep["thr2"]

    rhs = np.zeros((T, 5, Lmax), f32)
    # pad column: a real point (harmless extra candidate)
    rhs[:, 0, :] = pts[0, 0]
    rhs[:, 1, :] = pts[0, 1]
    rhs[:, 2, :] = pts[0, 2]
    rhs[:, 3, :] = 1.0
    rhs[:, 4, :] = -psq[0]
    wts = np.zeros((T, 5, N), f32)

    for s in range(T):
        t = prep["plane_rank"][s]
        og = ords[t]
        # weights: blocks permuted into slot order
        pos = 0
        for j in range(NBLK):
            k = prep["blk_rank"][t][j]
            rows = og[k * 128:(k + 1) * 128]
            wts[s, 0, j * 128:(j + 1) * 128] = 2.0 * refl[t, rows, 0]
            wts[s, 1, j * 128:(j + 1) * 128] = 2.0 * refl[t, rows, 1]
            wts[s, 2, j * 128:(j + 1) * 128] = 2.0 * refl[t, rows, 2]
            wts[s, 3, j * 128:(j + 1) * 128] = thr2 - rsq[t, rows]
            wts[s, 4, j * 128:(j + 1) * 128] = 1.0
            # candidates for (t,k): m rows og[lo : lo+width]
            w_have = int(prep["widths"][t][k])
            w_slot = int(W_union[s][j])
            lo = int(prep["los"][t][k])
            cand = og[lo:lo + w_have]
            rhs[s, 0, pos:pos + w_have] = pts[cand, 0]
            rhs[s, 1, pos:pos + w_have] = pts[cand, 1]
            rhs[s, 2, pos:pos + w_have] = pts[cand, 2]
            rhs[s, 3, pos:pos + w_have] = 1.0
            rhs[s, 4, pos:pos + w_have] = -psq[cand]
            pos += w_slot
    return rhs, wts


def _host_finish(prep, maxv, col_map):
    """maxv [128, ncols] -> per-plane counts -> selection outputs."""
    f32 = np.float32
    # unit max over chunk columns
    umax = {}
    for ci, (s, j, lo, cw) in enumerate(col_map):
        cur = umax.get((s, j))
        col = maxv[:, ci]
        umax[(s, j)] = col if cur is None else np.maximum(cur, col)

    pts, psq = prep["pts"], prep["psq"]
    refl, rsq = prep["refl"], prep["rsq"]
    counts = np.zeros(T, np.int64)
    for s in range(T):
        t = prep["plane_rank"][s]
        og = prep["ords"][t]
        c = 0
        for j in range(NBLK):
            k = prep["blk_rank"][t][j]
            mv = umax.get((s, j))                  # [128]
            if mv is None:
                continue  # empty interval in every batch: no m can match
            ind = mv >= 0.0
            bad = np.nonzero(np.abs(mv) < BORDER_EPS)[0]
            if len(bad):
                rows = og[k * 128 + bad]
                for bi_, nrow in zip(bad, rows):
                    cross = pts @ refl[t, nrow]
                    d2 = rsq[t, nrow] + psq - f32(2.0) * cross
                    md = np.sqrt(np.maximum(d2, 0.0)).min()
                    ind[bi_] = md < MATCH_THR
            c += int(ind.sum())
        counts[t] = c

    frac = counts.astype(f32) / f32(N)
    frac = np.where(prep["nd"] < 1e-8, f32(-1.0), frac)

    best_n = np.array([0.0, 1.0, 0.0], f32)
    best_o = f32(0.0)
    best_c = f32(0.0)
    for t in range(T):
        if frac[t] > best_c:
            best_n = prep["n"][t]
            best_o = prep["offset"][t]
            best_c = frac[t]
    return best_n.astype(f32), f32(best_o), f32(best_c)


def _ensure_axon_hooks():
    """The agent image's antenv lacks axon_hooks; bass_utils imports it
    unconditionally when trace is requested. Inject a functional stand-in
    and register the ctypes NTFF hook so profiling works."""
    import types
    if "antenv.axon_hooks" in sys.modules:
        return
    try:
        import antenv
        m = types.ModuleType("antenv.axon_hooks")
        m._hook = None
        m.set_axon_ntff_profile_hook = lambda h: setattr(m, "_hook", h)
        m.get_axon_ntff_profile_hook = lambda: m._hook
        sys.modules["antenv.axon_hooks"] = m
        antenv.axon_hooks = m
        from trn_agent_boot.trn_boot import _ntff_profile_via_ctypes
        m._hook = _ntff_profile_via_ctypes("/opt/axon/libaxon_pjrt.so")
    except Exception:
        pass


def kernel(points, z_enc, z_local, proxy_coords, sample_idx, W, b):
    from concourse.bass_utils import run_bass_kernel_spmd
    _ensure_axon_hooks()

    f32 = np.float32
    points = np.asarray(points)
    sample_idx = np.asarray(sample_idx)
    preps = [_host_prep(points[i], sample_idx[i]) for i in range(B)]

    W_union = np.stack([p["wslot"] for p in preps]).max(0)   # [T, NBLK]

    ck = tuple(map(int, W_union.ravel()))
    if ck not in _COMPILED:
        _COMPILED[ck] = (_build_bass(W_union), _chunk_plan(W_union))
    nc, (col_map, rhs_off, Lmax) = _COMPILED[ck]

    in_maps = []
    for i in range(B):
        rhs, wts = _host_operands(preps[i], W_union, Lmax)
        in_maps.append({"rhs": rhs, "wts": wts})
    res = run_bass_kernel_spmd(nc, in_maps, list(range(B)))
    globals()["_LAST_RESULT"] = res

    normals = np.zeros((B, 3), f32)
    offsets = np.zeros((B,), f32)
    confs = np.zeros((B,), f32)
    for i in range(B):
        bn, bo, bc = _host_finish(preps[i], res.results[i]["maxv"], col_map)
        normals[i], offsets[i], confs[i] = bn, bo, bc

    centroid = points.mean(axis=1, dtype=f32).astype(f32)
    signed_dist = (normals * centroid).sum(-1, dtype=f32) - offsets
    sym_feats = np.concatenate([normals, signed_dist[:, None]], -1)
    z_aug = np.concatenate([np.asarray(z_enc, f32), sym_feats], -1)
    z_enc_aug = (z_aug @ np.asarray(W, f32).T + np.asarray(b, f32)).astype(f32)
    return (z_enc_aug, np.asarray(z_local), normals, offsets, confs)
